# revision 32
# baseline (speedup 1.0000x reference)
"""Trainium2 Bass kernel for BEiT attention block (nn_Beit_9560597201107).

Data-parallel over batch: 64 batches -> 8 NeuronCores x 8 batches each.
Transposed dataflow (channels on partitions) so the softmax'd attention
matrix is never transposed on-chip; batch PAIRS are packed to 394 token
columns (>=256 keeps fp32r at 1 cycle/row) for the qkv and output
projections, and the post-exp path runs in bf16 (full rate at any free
size, so token dims shrink to the exact 197):

  qkT[c, 2x197] = sum_k WT[k, c] xT[k, :] + qbias     fp32r, pair-packed
  v[m, (sp, i, d|1)]  = sum_k xT[k, m] WT_v[k, d]     65th column = ones
  scT_h[m, n] = sum_d kT[d, m] qT[d, n]               fp32r; the two heads
      of a pair sit at partition halves 0:64 / 64:128 -> concurrent
      row-tiled matmuls into separate PSUM banks
  eT = exp(scT) * exp_rel_T                           bf16 [m, 2x197]
  poA[0:65, 2x197] = [v_h0 | 1]^T eT                  row 64 = softmax sums
  poB[64:128, 197] = v_h1^T eT_h1                     col-tiled to (0,64)
  pb = ones-outer-product broadcast of 1/sums         bf16 matmuls, N=197
  cT = po * pb;   yT[o, 2x197] = sum_c pwT[c, o] cT[c, :] + bias(partition)

Projection of pair g is emitted during pair g+1's qkT/v phases; scores are
software-pipelined one head-pair ahead so the PE never waits on the exp.
"""

import os
import numpy as np
from ml_dtypes import bfloat16

import concourse.bass as bass
import concourse.bacc as bacc
import concourse.mybir as mybir
import concourse.tile as tile
from concourse.bass_utils import run_bass_kernel_spmd
from concourse.bass_interp import get_hw_module

B, N, DIM, HEADS, NBS = 64, 197, 768, 12, 10
HEAD_DIM = DIM // HEADS
SCALE = HEAD_DIM ** -0.5
NCORES = 8
BPC = B // NCORES          # batches per core
NPAIR = BPC // 2
KT = DIM // 128            # 6 contraction tiles
N2 = 2 * N                 # 394: batch-pair-packed token columns
TOK_TILES = [(0, 128), (128, 69)]  # (offset, size) over the 197 tokens

F32 = mybir.dt.float32
F32R = mybir.dt.float32r
BF16 = mybir.dt.bfloat16
IDENT = mybir.ActivationFunctionType.Identity
EXP = mybir.ActivationFunctionType.Exp

_CACHE = {}


def _build_module():
    nc = bacc.Bacc("TRN2", target_bir_lowering=False, debug=False)

    xt_d = nc.dram_tensor("xt", [NPAIR, 128, KT, N2], F32, kind="ExternalInput")
    wtq_d = nc.dram_tensor("wtq", [6, 128, KT, 128], F32, kind="ExternalInput")
    wtk_d = nc.dram_tensor("wtk", [6, 128, KT, 128], F32, kind="ExternalInput")
    wtv_d = nc.dram_tensor("wtv", [128, KT, DIM], F32, kind="ExternalInput")
    pwt_d = nc.dram_tensor("pwt", [128, KT, DIM], BF16, kind="ExternalInput")
    relt_d = nc.dram_tensor("relt", [128, 6, 2, N2], BF16, kind="ExternalInput")
    qbc_d = nc.dram_tensor("qbc", [128, BPC, KT], F32, kind="ExternalInput")
    vpbt_d = nc.dram_tensor("vpbt", [128, KT, BPC], F32, kind="ExternalInput")
    aon_d = nc.dram_tensor("aon", [65, 128], BF16, kind="ExternalInput")
    von_d = nc.dram_tensor("von", [128, 12], BF16, kind="ExternalInput")
    yt_d = nc.dram_tensor("yt", [NPAIR, KT, 128, N2], F32, kind="ExternalOutput")
    dbg = os.environ.get("KERNEL_DEBUG", "0") == "1"
    if dbg:
        dqkT_d = nc.dram_tensor("dqkT", [128, 12, N2], F32, kind="ExternalOutput")
        dexp_d = nc.dram_tensor("dexp", [12, 128, 2, N2], F32, kind="ExternalOutput")
        dv_d = nc.dram_tensor("dv", [2, 128, 2, KT, 2, 65], F32, kind="ExternalOutput")
        dcT_d = nc.dram_tensor("dcT", [128, KT, N2], F32, kind="ExternalOutput")
        dpoA_d = nc.dram_tensor("dpoA", [128, 512], F32, kind="ExternalOutput")
        dpoB_d = nc.dram_tensor("dpoB", [128, 512], F32, kind="ExternalOutput")
        drec_d = nc.dram_tensor("drec", [1, N2], F32, kind="ExternalOutput")
        drecf_d = nc.dram_tensor("drecf", [1, N2], F32, kind="ExternalOutput")
        dpb_d = nc.dram_tensor("dpb", [128, N2], F32, kind="ExternalOutput")
        dpwt_d = nc.dram_tensor("dpwt", [128, KT, DIM], F32, kind="ExternalOutput")
        dcT2_d = nc.dram_tensor("dcT2", [128, KT, N2], F32, kind="ExternalOutput")

    with tile.TileContext(nc) as tc:
        with (
            tc.tile_pool(name="const", bufs=1) as constp,
            tc.tile_pool(name="sb_xT", bufs=2) as sb_xT,
            tc.tile_pool(name="sb_qkT", bufs=2) as sb_qkT,
            tc.tile_pool(name="sb_v", bufs=2) as sb_v,
            tc.tile_pool(name="sb_exp", bufs=3) as sb_exp,
            tc.tile_pool(name="sb_rec", bufs=6) as sb_rec,
            tc.tile_pool(name="sb_pb", bufs=4) as sb_pb,
            tc.tile_pool(name="sb_cT", bufs=2) as sb_cT,
            tc.tile_pool(name="sb_out", bufs=3) as sb_out,
            tc.tile_pool(name="ps", bufs=4, space="PSUM") as ps,
            tc.tile_pool(name="ps2", bufs=2, space="PSUM") as ps2,
        ):
            # ---- persistent data, streamed in consumption order ----
            # gpsimd queue (fp32 -> fp32r casting loads): wtq ct0, x pair0,
            # wtq rest, wtk, wtv, x pair1. sync queue (no cast): the small
            # bf16/f32 tables, racing ahead in parallel.
            wtq_sb = constp.tile([128, 6, KT, 128], F32R)
            wtk_sb = constp.tile([128, 6, KT, 128], F32R)
            nc.gpsimd.dma_start(out=wtq_sb[:, 0], in_=wtq_d.ap()[0])

            xT_tiles = {}

            def load_xT(g):
                t_ = sb_xT.tile([128, KT, N2], F32R, tag="xT", name=f"xT_{g}")
                nc.gpsimd.dma_start(out=t_[:], in_=xt_d.ap()[g])
                xT_tiles[g] = t_

            load_xT(0)
            for ct in range(1, 6):
                nc.gpsimd.dma_start(out=wtq_sb[:, ct], in_=wtq_d.ap()[ct])
            for ct in range(6):
                nc.gpsimd.dma_start(out=wtk_sb[:, ct], in_=wtk_d.ap()[ct])
            wtv_sb = constp.tile([128, KT, DIM], F32R)
            nc.gpsimd.dma_start(out=wtv_sb[:], in_=wtv_d.ap())
            load_xT(1)

            qbc_sb = constp.tile([128, BPC, KT], F32)
            nc.sync.dma_start(out=qbc_sb[:], in_=qbc_d.ap())
            vpbt_sb = constp.tile([128, KT, BPC], F32)
            nc.sync.dma_start(out=vpbt_sb[:], in_=vpbt_d.ap())
            aon_sb = constp.tile([65, 128], BF16)
            nc.sync.dma_start(out=aon_sb[:], in_=aon_d.ap())
            von_sb = constp.tile([128, 12], BF16)
            nc.sync.dma_start(out=von_sb[:], in_=von_d.ap())
            relt_sb = constp.tile([128, 6, 2, N2], BF16)
            nc.sync.dma_start(out=relt_sb[:], in_=relt_d.ap())
            pwt_sb = constp.tile([128, KT, DIM], BF16)
            nc.sync.dma_start(out=pwt_sb[:], in_=pwt_d.ap())

            # ---- transposed pair-packed output projection ----
            def emit_projT(src, o, c0, c1):
                gs, cT_ = src
                w = c1 - c0
                prT = ps.tile([128, 512], F32, tag="ps",
                              name=f"prT_{gs}_{o}_{c0}")
                for c in range(KT):
                    nc.tensor.matmul(
                        prT[:, 0:w], pwt_sb[:, c, o * 128:(o + 1) * 128],
                        cT_[:, c, c0:c1], start=(c == 0), stop=(c == KT - 1),
                    )
                out_sb = sb_out.tile([128, N2], F32, tag="out",
                                     name=f"out_{gs}_{o}_{c0}")
                if c0 == 0:
                    nc.scalar.activation(out_sb[:, 0:N], prT[:, 0:N], IDENT,
                                         bias=vpbt_sb[:, o, 2 * gs:2 * gs + 1])
                if c1 == N2:
                    nc.vector.tensor_scalar_add(
                        out_sb[:, N:N2], prT[:, w - N:w],
                        vpbt_sb[:, o, 2 * gs + 1:2 * gs + 2])
                nc.sync.dma_start(out=yt_d.ap()[gs, o, :, c0:c1],
                                  in_=out_sb[:, c0:c1])

            prev = [None]

            for g in range(NPAIR):
                xT_sb = xT_tiles.pop(g)

                # ---- qkT for the pair: 12 col-chunks x 6 k, N=394 fp32r ----
                qkT_sb = sb_qkT.tile([128, 12, N2], F32R, tag="qkT",
                                     name=f"qkT_{g}")
                for ct in range(12):
                    w = wtq_sb if ct < 6 else wtk_sb
                    qp = ps.tile([128, 512], F32, tag="ps", name=f"qp_{g}_{ct}")
                    for k in range(KT):
                        nc.tensor.matmul(
                            qp[:, 0:N2], w[:, ct % 6, k, :], xT_sb[:, k, :],
                            start=(k == 0), stop=(k == KT - 1),
                        )
                    for hb in range(2):
                        dst = qkT_sb[:, ct, hb * N:(hb + 1) * N]
                        src = qp[:, hb * N:(hb + 1) * N]
                        if ct < 6:
                            qb = qbc_sb[:, 2 * g + hb, ct:ct + 1]
                            if hb == 0:
                                nc.vector.tensor_scalar_add(dst, src, qb)
                            else:
                                nc.scalar.activation(dst, src, IDENT, bias=qb)
                        else:
                            if hb == 0:
                                nc.vector.tensor_copy(dst, src)
                            else:
                                nc.scalar.copy(dst, src)

                if dbg and g == 0:
                    nc.gpsimd.dma_start(out=dqkT_d.ap(), in_=qkT_sb[:, :, :])

                if g + 2 < NPAIR:
                    load_xT(g + 2)

                cT_pair = sb_cT.tile([128, KT, N2], BF16, tag="cT", name=f"cT_{g}")
                v_tiles = {}

                def v_phase(hb):
                    b = 2 * g + hb
                    v_sb = sb_v.tile([128, 2, KT, 2, 65], BF16, tag="v",
                                     name=f"v_{b}")
                    v_tiles[hb] = v_sb
                    for t, (off, mt) in enumerate(TOK_TILES):
                        nc.gpsimd.tensor_copy(
                            v_sb[:, t, :, :, 64:65],
                            von_sb[:, 0:12].rearrange("p (a i o) -> p a i o",
                                                      i=2, o=1),
                        )
                        vp = ps.tile([128, 512], F32, tag="ps", name=f"vp_{b}_{t}")
                        vp2 = ps.tile([128, 512], F32, tag="ps", name=f"vp2_{b}_{t}")
                        for k in range(KT):
                            xsl = xT_sb[:, k, hb * N + off:hb * N + off + mt]
                            nc.tensor.matmul(
                                vp[0:mt, 0:512], xsl, wtv_sb[:, k, 0:512],
                                start=(k == 0), stop=(k == KT - 1),
                            )
                            nc.tensor.matmul(
                                vp2[0:mt, 0:256], xsl, wtv_sb[:, k, 512:768],
                                start=(k == 0), stop=(k == KT - 1),
                            )
                        nc.vector.tensor_copy(
                            v_sb[0:mt, t, 0:4, 0:2, 0:64],
                            vp[0:mt, 0:512].rearrange(
                                "p (a i d) -> p a i d", i=2, d=64),
                        )
                        nc.scalar.copy(
                            v_sb[0:mt, t, 4:6, 0:2, 0:64],
                            vp2[0:mt, 0:256].rearrange(
                                "p (a i d) -> p a i d", i=2, d=64),
                        )
                    if dbg and g == 0:
                        nc.gpsimd.dma_start(out=dv_d.ap()[hb], in_=v_sb[:])

                def attn_phase(hb, interleave=None):
                    b = 2 * g + hb
                    v_sb = v_tiles[hb]
                    sc_tiles = {}

                    q0 = hb * 138   # query window start: [0,256) or [138,394)
                    sk = hb * 59    # in-window offset of this batch's queries

                    def emit_sc(sp):
                        sc = ps2.tile([128, 1024], F32, tag="ps2",
                                      name=f"sc_{b}_{sp}")
                        for t, (off, mt) in enumerate(TOK_TILES):
                            nc.tensor.matmul(
                                sc[0:mt, t * 256:t * 256 + 256],
                                qkT_sb[0:64, 6 + sp, hb * N + off:hb * N + off + mt],
                                qkT_sb[0:64, sp, q0:q0 + 256],
                                start=True, stop=True,
                            )
                            nc.tensor.matmul(
                                sc[0:mt, 512 + t * 256:512 + t * 256 + 256],
                                qkT_sb[64:128, 6 + sp, hb * N + off:hb * N + off + mt],
                                qkT_sb[64:128, sp, q0:q0 + 256],
                                start=True, stop=True,
                            )
                        sc_tiles[sp] = sc

                    emit_sc(0)
                    for sp in range(6):
                        # scores one pair ahead: PE computes sp+1's scores
                        # while the scalar engine runs sp's exp
                        if sp + 1 < 6:
                            emit_sc(sp + 1)
                        sc = sc_tiles.pop(sp)
                        expT = sb_exp.tile([128, 2, N2], BF16, tag="expT",
                                           name=f"expT_{b}_{sp}")
                        for t, (off, mt) in enumerate(TOK_TILES):
                            nc.scalar.activation(
                                expT[0:mt, t, :].rearrange(
                                    "p (h n) -> p h n", h=2),
                                sc[0:mt, :].rearrange(
                                    "p (h x) -> p h x", h=2)
                                [:, :, t * 256 + sk:t * 256 + sk + N], EXP)
                            nc.gpsimd.tensor_mul(
                                expT[0:mt, t, :], expT[0:mt, t, :],
                                relt_sb[0:mt, sp, t, :],
                            )
                        if dbg and g == 0:
                            nc.gpsimd.dma_start(out=dexp_d.ap()[hb * 6 + sp],
                                                in_=expT[:])
                        if interleave is not None:
                            interleave(sp)
                        poA = ps.tile([128, 512], F32, tag="ps", name=f"poA_{b}_{sp}")
                        poB = ps.tile([128, 512], F32, tag="ps", name=f"poB_{b}_{sp}")
                        for t, (off, mt) in enumerate(TOK_TILES):
                            nc.tensor.matmul(
                                poA[0:65, 0:N2], v_sb[0:mt, t, sp, 0, 0:65],
                                expT[0:mt, t, :], start=(t == 0), stop=(t == 1),
                            )
                            nc.tensor.matmul(
                                poB[64:128, 0:N], v_sb[0:mt, t, sp, 1, 0:64],
                                expT[0:mt, t, N:N2],
                                start=(t == 0), stop=(t == 1),
                            )
                        sums_b = sb_rec.tile([65, N2], BF16, tag="sums",
                                             name=f"sums_{b}_{sp}")
                        nc.scalar.copy(sums_b[64:65, :], poA[64:65, 0:N2])
                        pb = ps.tile([128, 512], F32, tag="ps", name=f"pb_{b}_{sp}")
                        nc.tensor.matmul(pb[0:128, 0:N2], aon_sb[64:65, 0:128],
                                         sums_b[64:65, 0:N2], start=True, stop=True)
                        pb_sb = sb_pb.tile([128, N2], F32, tag="pb",
                                           name=f"pb_{b}_{sp}")
                        nc.vector.reciprocal_approx_fast(
                            out=pb_sb[0:128, :], in_=pb[0:128, 0:N2])
                        if dbg and b == 0 and sp == 0:
                            tA = sb_out.tile([128, 512], F32, name="dbg_tA")
                            nc.vector.tensor_copy(tA[:], poA[:, :])
                            nc.gpsimd.dma_start(out=dpoA_d.ap(), in_=tA[:])
                            tB = sb_out.tile([128, 512], F32, name="dbg_tB")
                            nc.vector.tensor_copy(tB[:], poB[:, :])
                            nc.gpsimd.dma_start(out=dpoB_d.ap(), in_=tB[:])
                            nc.gpsimd.dma_start(out=dpb_d.ap(), in_=pb_sb[:])
                        nc.vector.tensor_mul(
                            cT_pair[0:64, sp, hb * N:(hb + 1) * N],
                            poA[0:64, 0:N], pb_sb[0:64, 0:N])
                        nc.vector.tensor_mul(
                            cT_pair[64:128, sp, hb * N:(hb + 1) * N],
                            poB[64:128, 0:N], pb_sb[64:128, N:N2])

                v_phase(0)
                if prev[0] is not None:
                    for o in range(3):
                        emit_projT(prev[0], o, 0, N2)
                attn_phase(0)
                v_phase(1)
                if prev[0] is not None:
                    for o in range(3, 6):
                        emit_projT(prev[0], o, 0, N2)
                if g == NPAIR - 1:
                    # tail: batch A's projection interleaves into batch B's
                    # attention; only batch B's half drains at the end
                    attn_phase(1, interleave=lambda sp: emit_projT(
                        (g, cT_pair), sp, 0, N))
                else:
                    attn_phase(1)
                if dbg and g == 0:
                    nc.gpsimd.dma_start(out=dcT_d.ap(), in_=cT_pair[:])
                prev[0] = (g, cT_pair)

            for o in range(KT):
                emit_projT(prev[0], o, N, N2)
            if dbg:
                nc.gpsimd.dma_start(out=dpwt_d.ap(), in_=pwt_sb[:])

    nc.compile()
    nc.m = get_hw_module(nc.m)
    return nc


def _host_prep(x, qkv_weight, q_bias, v_bias, rel_table, proj_weight, proj_bias,
               b_idx, rel_index):
    x = np.asarray(x, dtype=np.float32)
    W = np.asarray(qkv_weight, dtype=np.float32).copy()
    W[:DIM] *= np.float32(SCALE)
    WT = np.ascontiguousarray(W.T)               # [cin, cout]
    wtq = np.ascontiguousarray(
        WT[:, 0:DIM].reshape(KT, 128, 6, 128).transpose(2, 1, 0, 3))
    wtk = np.ascontiguousarray(
        WT[:, DIM:2 * DIM].reshape(KT, 128, 6, 128).transpose(2, 1, 0, 3))
    wtv = np.ascontiguousarray(
        WT[:, 2 * DIM:].reshape(KT, 128, DIM).transpose(1, 0, 2))
    pwtT = np.asarray(proj_weight, dtype=np.float32).T   # [cin, cout]
    pwt = np.ascontiguousarray(
        pwtT.reshape(KT, 128, DIM).transpose(1, 0, 2)).astype(bfloat16)

    bi = np.asarray(b_idx).astype(np.int64)
    qb_all = np.asarray(q_bias, dtype=np.float32)[bi] * np.float32(SCALE)
    vb_all = np.asarray(v_bias, dtype=np.float32)[bi]
    # softmax rows sum to 1, so attn @ (1 x vb) == 1 x vb; push the v bias
    # through the projection into the proj bias
    pb_all = (np.asarray(proj_bias, dtype=np.float32)[bi]
              + vb_all @ np.asarray(proj_weight, dtype=np.float32).T)

    ridx = np.asarray(rel_index).astype(np.int64)
    relE = np.exp(np.asarray(rel_table, dtype=np.float32)[ridx.reshape(-1)]
                  .reshape(N, N, HEADS))           # [n, m, h]
    relM = relE.transpose(1, 0, 2)                  # [m, n, h]
    relt = np.zeros((128, 6, 2, N2), dtype=np.float32)
    for t, (off, mt) in enumerate(TOK_TILES):
        seg = relM[off:off + mt]                    # [mt, n, h]
        relt[0:mt, :, t, :] = (seg.reshape(mt, N, 6, 2)
                               .transpose(0, 2, 3, 1).reshape(mt, 6, N2))
    relt = relt.astype(bfloat16)
    aon = np.ones((65, 128), dtype=bfloat16)
    von = np.ones((128, 12), dtype=bfloat16)

    in_maps = []
    for c in range(NCORES):
        sl = slice(c * BPC, (c + 1) * BPC)
        xs = x[sl]                                  # [8, 197, 768]
        xt = np.ascontiguousarray(
            xs.reshape(NPAIR, 2, N, DIM).transpose(0, 3, 1, 2)
            .reshape(NPAIR, KT, 128, N2).transpose(0, 2, 1, 3))
        qbc = np.ascontiguousarray(
            qb_all[sl].reshape(BPC, KT, 128).transpose(2, 0, 1))
        vpbt = np.ascontiguousarray(
            pb_all[sl].reshape(BPC, KT, 128).transpose(2, 1, 0))
        in_maps.append({
            "xt": xt,
            "wtq": wtq,
            "wtk": wtk,
            "wtv": wtv,
            "pwt": pwt,
            "relt": relt,
            "qbc": qbc,
            "vpbt": vpbt,
            "aon": aon,
            "von": von,
        })
    return in_maps


def _install_ntff_hook():
    """Provide antenv.axon_hooks (absent from this image) so bass_utils can
    capture NTFF profiles through libaxon_pjrt.so, and keep artifacts local."""
    if _CACHE.get("hook_installed"):
        return
    import sys
    import types
    import ctypes
    import contextlib

    so_path = "/opt/axon/libaxon_pjrt.so"
    lib = ctypes.CDLL(so_path)
    lib.axon_start_nrt_profile.argtypes = [
        ctypes.POINTER(ctypes.c_int64),
        ctypes.c_size_t,
    ]
    lib.axon_start_nrt_profile.restype = ctypes.c_int64
    lib.axon_stop_nrt_profile.argtypes = [ctypes.c_char_p]
    lib.axon_stop_nrt_profile.restype = ctypes.c_int64

    @contextlib.contextmanager
    def _hook(output_dir, device_ids):
        import jax

        jax.devices()
        if device_ids:
            ids = (ctypes.c_int64 * len(device_ids))(*device_ids)
            rc = lib.axon_start_nrt_profile(ids, len(device_ids))
        else:
            rc = lib.axon_start_nrt_profile(None, 0)
        if rc != 0:
            raise RuntimeError(f"axon_start_nrt_profile rc={rc}")
        try:
            yield
        finally:
            n = lib.axon_stop_nrt_profile(str(output_dir).encode())
            print(f"ntff profile: {n} file(s) written to {output_dir}")

    mod = types.ModuleType("antenv.axon_hooks")
    mod.get_axon_ntff_profile_hook = lambda: _hook
    mod.set_axon_ntff_profile_hook = lambda h: None
    sys.modules["antenv.axon_hooks"] = mod

    import concourse.bass_utils as bu

    bu.upload_artifacts = lambda tmpdir: str(tmpdir)
    _CACHE["hook_installed"] = True


def kernel(**inputs):
    if "nc" not in _CACHE:
        _CACHE["nc"] = _build_module()
    nc = _CACHE["nc"]

    in_maps = _host_prep(**inputs)
    trace = os.environ.get("KERNEL_TRACE", "0") == "1"
    tmpdir = None
    if trace:
        _install_ntff_hook()
        tmpdir = os.environ.get("KERNEL_TRACE_DIR") or None
    res = run_bass_kernel_spmd(nc, in_maps, core_ids=list(range(NCORES)), trace=trace,
                               tmpdir=tmpdir)
    if trace:
        _CACHE["last_exec_time_ns"] = res.exec_time_ns
        _CACHE["last_results"] = res

    ys = []
    for c in range(NCORES):
        yt = np.asarray(res.results[c]["yt"])       # [4, 6, 128, 394]
        ys.append(yt.reshape(NPAIR, KT, 128, 2, N)
                  .transpose(0, 3, 4, 1, 2).reshape(BPC, N, DIM))
    return np.ascontiguousarray(np.concatenate(ys, axis=0), dtype=np.float32)


# revision 34
# speedup vs baseline: 1.0394x; 1.0394x over previous
"""Trainium2 Bass kernel for BEiT attention block (nn_Beit_9560597201107).

Data-parallel over batch: 64 batches -> 8 NeuronCores x 8 batches each.
Transposed dataflow (channels on partitions) so the softmax'd attention
matrix is never transposed on-chip; batch PAIRS are packed to 394 token
columns (>=256 keeps fp32r at 1 cycle/row) for the qkv and output
projections, and the post-exp path runs in bf16 (full rate at any free
size, so token dims shrink to the exact 197):

  qkT[c, 2x197] = sum_k WT[k, c] xT[k, :] + qbias     fp32r, pair-packed
  v[m, (sp, i, d|1)]  = sum_k xT[k, m] WT_v[k, d]     65th column = ones
  scT_h[m, n] = sum_d kT[d, m] qT[d, n]               fp32r; the two heads
      of a pair sit at partition halves 0:64 / 64:128 -> concurrent
      row-tiled matmuls into separate PSUM banks
  eT = exp(scT) * exp_rel_T                           bf16 [m, 2x197]
  poA[0:65, 2x197] = [v_h0 | 1]^T eT                  row 64 = softmax sums
  poB[64:128, 197] = v_h1^T eT_h1                     col-tiled to (0,64)
  pb = ones-outer-product broadcast of 1/sums         bf16 matmuls, N=197
  cT = po * pb;   yT[o, 2x197] = sum_c pwT[c, o] cT[c, :] + bias(partition)

Projection of pair g is emitted during pair g+1's qkT/v phases; scores are
software-pipelined one head-pair ahead so the PE never waits on the exp.
"""

import os
import numpy as np
from ml_dtypes import bfloat16

import concourse.bass as bass
import concourse.bacc as bacc
import concourse.mybir as mybir
import concourse.tile as tile
from concourse.bass_utils import run_bass_kernel_spmd
from concourse.bass_interp import get_hw_module

B, N, DIM, HEADS, NBS = 64, 197, 768, 12, 10
HEAD_DIM = DIM // HEADS
SCALE = HEAD_DIM ** -0.5
NCORES = 8
BPC = B // NCORES          # batches per core
NPAIR = BPC // 2
KT = DIM // 128            # 6 contraction tiles
N2 = 2 * N                 # 394: batch-pair-packed token columns
TOK_TILES = [(0, 128), (128, 69)]  # (offset, size) over the 197 tokens

F32 = mybir.dt.float32
F32R = mybir.dt.float32r
BF16 = mybir.dt.bfloat16
IDENT = mybir.ActivationFunctionType.Identity
EXP = mybir.ActivationFunctionType.Exp

_CACHE = {}


def _build_module():
    nc = bacc.Bacc("TRN2", target_bir_lowering=False, debug=False)

    xt_d = nc.dram_tensor("xt", [NPAIR, 128, KT, N2], F32, kind="ExternalInput")
    wtq_d = nc.dram_tensor("wtq", [6, 128, KT, 128], F32, kind="ExternalInput")
    wtk_d = nc.dram_tensor("wtk", [6, 128, KT, 128], F32, kind="ExternalInput")
    wtv_d = nc.dram_tensor("wtv", [128, KT, DIM], F32, kind="ExternalInput")
    pwt_d = nc.dram_tensor("pwt", [128, KT, DIM], BF16, kind="ExternalInput")
    relt_d = nc.dram_tensor("relt", [128, 6, 2, N2], BF16, kind="ExternalInput")
    qbc_d = nc.dram_tensor("qbc", [128, BPC, KT], F32, kind="ExternalInput")
    vpbt_d = nc.dram_tensor("vpbt", [128, KT, BPC], F32, kind="ExternalInput")
    aon_d = nc.dram_tensor("aon", [65, 128], BF16, kind="ExternalInput")
    von_d = nc.dram_tensor("von", [128, 12], BF16, kind="ExternalInput")
    yt_d = nc.dram_tensor("yt", [NPAIR, KT, 128, N2], F32, kind="ExternalOutput")
    dbg = os.environ.get("KERNEL_DEBUG", "0") == "1"
    if dbg:
        dqkT_d = nc.dram_tensor("dqkT", [128, 12, N2], F32, kind="ExternalOutput")
        dexp_d = nc.dram_tensor("dexp", [12, 128, 2, N2], F32, kind="ExternalOutput")
        dv_d = nc.dram_tensor("dv", [2, 128, 2, KT, 2, 65], F32, kind="ExternalOutput")
        dcT_d = nc.dram_tensor("dcT", [128, KT, N2], F32, kind="ExternalOutput")
        dpoA_d = nc.dram_tensor("dpoA", [128, 512], F32, kind="ExternalOutput")
        dpoB_d = nc.dram_tensor("dpoB", [128, 512], F32, kind="ExternalOutput")
        drec_d = nc.dram_tensor("drec", [1, N2], F32, kind="ExternalOutput")
        drecf_d = nc.dram_tensor("drecf", [1, N2], F32, kind="ExternalOutput")
        dpb_d = nc.dram_tensor("dpb", [128, N2], F32, kind="ExternalOutput")
        dpwt_d = nc.dram_tensor("dpwt", [128, KT, DIM], F32, kind="ExternalOutput")
        dcT2_d = nc.dram_tensor("dcT2", [128, KT, N2], F32, kind="ExternalOutput")

    with tile.TileContext(nc) as tc:
        with (
            tc.tile_pool(name="const", bufs=1) as constp,
            tc.tile_pool(name="sb_xT", bufs=2) as sb_xT,
            tc.tile_pool(name="sb_qkT", bufs=2) as sb_qkT,
            tc.tile_pool(name="sb_v", bufs=2) as sb_v,
            tc.tile_pool(name="sb_exp", bufs=3) as sb_exp,
            tc.tile_pool(name="sb_rec", bufs=6) as sb_rec,
            tc.tile_pool(name="sb_pb", bufs=4) as sb_pb,
            tc.tile_pool(name="sb_cT", bufs=2) as sb_cT,
            tc.tile_pool(name="sb_out", bufs=3) as sb_out,
            tc.tile_pool(name="ps", bufs=4, space="PSUM") as ps,
            tc.tile_pool(name="ps2", bufs=2, space="PSUM") as ps2,
        ):
            # ---- persistent data, streamed in consumption order ----
            # gpsimd queue (fp32 -> fp32r casting loads): wtq ct0, x pair0,
            # wtq rest, wtk, wtv, x pair1. sync queue (no cast): the small
            # bf16/f32 tables, racing ahead in parallel.
            wtq_sb = constp.tile([128, 6, KT, 128], F32R)
            wtk_sb = constp.tile([128, 6, KT, 128], F32R)
            nc.gpsimd.dma_start(out=wtq_sb[:, 0], in_=wtq_d.ap()[0])

            xT_tiles = {}

            def load_xT(g):
                t_ = sb_xT.tile([128, KT, N2], F32R, tag="xT", name=f"xT_{g}")
                nc.gpsimd.dma_start(out=t_[:], in_=xt_d.ap()[g])
                xT_tiles[g] = t_

            load_xT(0)
            for ct in range(1, 6):
                nc.gpsimd.dma_start(out=wtq_sb[:, ct], in_=wtq_d.ap()[ct])
            for ct in range(6):
                nc.gpsimd.dma_start(out=wtk_sb[:, ct], in_=wtk_d.ap()[ct])
            wtv_sb = constp.tile([128, KT, DIM], F32R)
            nc.gpsimd.dma_start(out=wtv_sb[:], in_=wtv_d.ap())
            load_xT(1)

            qbc_sb = constp.tile([128, BPC, KT], F32)
            nc.sync.dma_start(out=qbc_sb[:], in_=qbc_d.ap())
            vpbt_sb = constp.tile([128, KT, BPC], F32)
            nc.sync.dma_start(out=vpbt_sb[:], in_=vpbt_d.ap())
            aon_sb = constp.tile([65, 128], BF16)
            nc.sync.dma_start(out=aon_sb[:], in_=aon_d.ap())
            von_sb = constp.tile([128, 12], BF16)
            nc.sync.dma_start(out=von_sb[:], in_=von_d.ap())
            relt_sb = constp.tile([128, 6, 2, N2], BF16)
            nc.sync.dma_start(out=relt_sb[:], in_=relt_d.ap())
            pwt_sb = constp.tile([128, KT, DIM], BF16)
            nc.sync.dma_start(out=pwt_sb[:], in_=pwt_d.ap())

            # ---- transposed pair-packed output projection ----
            def emit_projT(src, o, c0, c1):
                gs, cT_ = src
                w = c1 - c0
                prT = ps.tile([128, 512], F32, tag="ps",
                              name=f"prT_{gs}_{o}_{c0}")
                for c in range(KT):
                    nc.tensor.matmul(
                        prT[:, 0:w], pwt_sb[:, c, o * 128:(o + 1) * 128],
                        cT_[:, c, c0:c1], start=(c == 0), stop=(c == KT - 1),
                    )
                out_sb = sb_out.tile([128, N2], F32, tag="out",
                                     name=f"out_{gs}_{o}_{c0}")
                if c0 == 0:
                    nc.scalar.activation(out_sb[:, 0:N], prT[:, 0:N], IDENT,
                                         bias=vpbt_sb[:, o, 2 * gs:2 * gs + 1])
                if c1 == N2:
                    nc.vector.tensor_scalar_add(
                        out_sb[:, N:N2], prT[:, w - N:w],
                        vpbt_sb[:, o, 2 * gs + 1:2 * gs + 2])
                nc.sync.dma_start(out=yt_d.ap()[gs, o, :, c0:c1],
                                  in_=out_sb[:, c0:c1])

            prev = [None]

            for g in range(NPAIR):
                xT_sb = xT_tiles.pop(g)

                # ---- qkT for the pair: 12 col-chunks x 6 k, N=394 fp32r ----
                qkT_sb = sb_qkT.tile([128, 12, N2], BF16, tag="qkT",
                                     name=f"qkT_{g}")
                for ct in range(12):
                    w = wtq_sb if ct < 6 else wtk_sb
                    qp = ps.tile([128, 512], F32, tag="ps", name=f"qp_{g}_{ct}")
                    for k in range(KT):
                        nc.tensor.matmul(
                            qp[:, 0:N2], w[:, ct % 6, k, :], xT_sb[:, k, :],
                            start=(k == 0), stop=(k == KT - 1),
                        )
                    for hb in range(2):
                        dst = qkT_sb[:, ct, hb * N:(hb + 1) * N]
                        src = qp[:, hb * N:(hb + 1) * N]
                        if ct < 6:
                            qb = qbc_sb[:, 2 * g + hb, ct:ct + 1]
                            if hb == 0:
                                nc.vector.tensor_scalar_add(dst, src, qb)
                            else:
                                nc.scalar.activation(dst, src, IDENT, bias=qb)
                        else:
                            if hb == 0:
                                nc.vector.tensor_copy(dst, src)
                            else:
                                nc.scalar.copy(dst, src)

                if dbg and g == 0:
                    nc.gpsimd.dma_start(out=dqkT_d.ap(), in_=qkT_sb[:, :, :])

                if g + 2 < NPAIR:
                    load_xT(g + 2)

                cT_pair = sb_cT.tile([128, KT, N2], BF16, tag="cT", name=f"cT_{g}")
                v_tiles = {}

                def v_phase(hb):
                    b = 2 * g + hb
                    v_sb = sb_v.tile([128, 2, KT, 2, 65], BF16, tag="v",
                                     name=f"v_{b}")
                    v_tiles[hb] = v_sb
                    for t, (off, mt) in enumerate(TOK_TILES):
                        nc.gpsimd.tensor_copy(
                            v_sb[:, t, :, :, 64:65],
                            von_sb[:, 0:12].rearrange("p (a i o) -> p a i o",
                                                      i=2, o=1),
                        )
                        vp = ps.tile([128, 512], F32, tag="ps", name=f"vp_{b}_{t}")
                        vp2 = ps.tile([128, 512], F32, tag="ps", name=f"vp2_{b}_{t}")
                        for k in range(KT):
                            xsl = xT_sb[:, k, hb * N + off:hb * N + off + mt]
                            nc.tensor.matmul(
                                vp[0:mt, 0:512], xsl, wtv_sb[:, k, 0:512],
                                start=(k == 0), stop=(k == KT - 1),
                            )
                            nc.tensor.matmul(
                                vp2[0:mt, 0:256], xsl, wtv_sb[:, k, 512:768],
                                start=(k == 0), stop=(k == KT - 1),
                            )
                        nc.vector.tensor_copy(
                            v_sb[0:mt, t, 0:4, 0:2, 0:64],
                            vp[0:mt, 0:512].rearrange(
                                "p (a i d) -> p a i d", i=2, d=64),
                        )
                        nc.scalar.copy(
                            v_sb[0:mt, t, 4:6, 0:2, 0:64],
                            vp2[0:mt, 0:256].rearrange(
                                "p (a i d) -> p a i d", i=2, d=64),
                        )
                    if dbg and g == 0:
                        nc.gpsimd.dma_start(out=dv_d.ap()[hb], in_=v_sb[:])

                def attn_phase(hb, interleave=None):
                    b = 2 * g + hb
                    v_sb = v_tiles[hb]
                    sc_tiles = {}


                    def emit_sc(sp):
                        sc = ps2.tile([128, 1024], F32, tag="ps2",
                                      name=f"sc_{b}_{sp}")
                        for t, (off, mt) in enumerate(TOK_TILES):
                            nc.tensor.matmul(
                                sc[0:mt, t * 256:t * 256 + N],
                                qkT_sb[0:64, 6 + sp, hb * N + off:hb * N + off + mt],
                                qkT_sb[0:64, sp, hb * N:(hb + 1) * N],
                                start=True, stop=True,
                            )
                            nc.tensor.matmul(
                                sc[0:mt, 512 + t * 256:512 + t * 256 + N],
                                qkT_sb[64:128, 6 + sp, hb * N + off:hb * N + off + mt],
                                qkT_sb[64:128, sp, hb * N:(hb + 1) * N],
                                start=True, stop=True,
                            )
                        sc_tiles[sp] = sc

                    emit_sc(0)
                    for sp in range(6):
                        # scores one pair ahead: PE computes sp+1's scores
                        # while the scalar engine runs sp's exp
                        if sp + 1 < 6:
                            emit_sc(sp + 1)
                        sc = sc_tiles.pop(sp)
                        expT = sb_exp.tile([128, 2, N2], BF16, tag="expT",
                                           name=f"expT_{b}_{sp}")
                        for t, (off, mt) in enumerate(TOK_TILES):
                            nc.scalar.activation(
                                expT[0:mt, t, :].rearrange(
                                    "p (h n) -> p h n", h=2),
                                sc[0:mt, :].rearrange(
                                    "p (h x) -> p h x", h=2)
                                [:, :, t * 256:t * 256 + N], EXP)
                            nc.gpsimd.tensor_mul(
                                expT[0:mt, t, :], expT[0:mt, t, :],
                                relt_sb[0:mt, sp, t, :],
                            )
                        if dbg and g == 0:
                            nc.gpsimd.dma_start(out=dexp_d.ap()[hb * 6 + sp],
                                                in_=expT[:])
                        if interleave is not None:
                            interleave(sp)
                        poA = ps.tile([128, 512], F32, tag="ps", name=f"poA_{b}_{sp}")
                        poB = ps.tile([128, 512], F32, tag="ps", name=f"poB_{b}_{sp}")
                        for t, (off, mt) in enumerate(TOK_TILES):
                            nc.tensor.matmul(
                                poA[0:65, 0:N2], v_sb[0:mt, t, sp, 0, 0:65],
                                expT[0:mt, t, :], start=(t == 0), stop=(t == 1),
                            )
                            nc.tensor.matmul(
                                poB[64:128, 0:N], v_sb[0:mt, t, sp, 1, 0:64],
                                expT[0:mt, t, N:N2],
                                start=(t == 0), stop=(t == 1),
                            )
                        sums_b = sb_rec.tile([65, N2], BF16, tag="sums",
                                             name=f"sums_{b}_{sp}")
                        nc.scalar.copy(sums_b[64:65, :], poA[64:65, 0:N2])
                        pb = ps.tile([128, 512], F32, tag="ps", name=f"pb_{b}_{sp}")
                        nc.tensor.matmul(pb[0:128, 0:N2], aon_sb[64:65, 0:128],
                                         sums_b[64:65, 0:N2], start=True, stop=True)
                        pb_sb = sb_pb.tile([128, N2], F32, tag="pb",
                                           name=f"pb_{b}_{sp}")
                        nc.vector.reciprocal_approx_fast(
                            out=pb_sb[0:128, :], in_=pb[0:128, 0:N2])
                        if dbg and b == 0 and sp == 0:
                            tA = sb_out.tile([128, 512], F32, name="dbg_tA")
                            nc.vector.tensor_copy(tA[:], poA[:, :])
                            nc.gpsimd.dma_start(out=dpoA_d.ap(), in_=tA[:])
                            tB = sb_out.tile([128, 512], F32, name="dbg_tB")
                            nc.vector.tensor_copy(tB[:], poB[:, :])
                            nc.gpsimd.dma_start(out=dpoB_d.ap(), in_=tB[:])
                            nc.gpsimd.dma_start(out=dpb_d.ap(), in_=pb_sb[:])
                        nc.vector.tensor_mul(
                            cT_pair[0:64, sp, hb * N:(hb + 1) * N],
                            poA[0:64, 0:N], pb_sb[0:64, 0:N])
                        nc.vector.tensor_mul(
                            cT_pair[64:128, sp, hb * N:(hb + 1) * N],
                            poB[64:128, 0:N], pb_sb[64:128, N:N2])

                v_phase(0)
                if prev[0] is not None:
                    for o in range(3):
                        emit_projT(prev[0], o, 0, N2)
                attn_phase(0)
                v_phase(1)
                if prev[0] is not None:
                    for o in range(3, 6):
                        emit_projT(prev[0], o, 0, N2)
                if g == NPAIR - 1:
                    # tail: batch A's projection interleaves into batch B's
                    # attention; only batch B's half drains at the end
                    attn_phase(1, interleave=lambda sp: emit_projT(
                        (g, cT_pair), sp, 0, N))
                else:
                    attn_phase(1)
                if dbg and g == 0:
                    nc.gpsimd.dma_start(out=dcT_d.ap(), in_=cT_pair[:])
                prev[0] = (g, cT_pair)

            for o in range(KT):
                emit_projT(prev[0], o, N, N2)
            if dbg:
                nc.gpsimd.dma_start(out=dpwt_d.ap(), in_=pwt_sb[:])

    nc.compile()
    nc.m = get_hw_module(nc.m)
    return nc


def _host_prep(x, qkv_weight, q_bias, v_bias, rel_table, proj_weight, proj_bias,
               b_idx, rel_index):
    x = np.asarray(x, dtype=np.float32)
    W = np.asarray(qkv_weight, dtype=np.float32).copy()
    W[:DIM] *= np.float32(SCALE)
    WT = np.ascontiguousarray(W.T)               # [cin, cout]
    wtq = np.ascontiguousarray(
        WT[:, 0:DIM].reshape(KT, 128, 6, 128).transpose(2, 1, 0, 3))
    wtk = np.ascontiguousarray(
        WT[:, DIM:2 * DIM].reshape(KT, 128, 6, 128).transpose(2, 1, 0, 3))
    wtv = np.ascontiguousarray(
        WT[:, 2 * DIM:].reshape(KT, 128, DIM).transpose(1, 0, 2))
    pwtT = np.asarray(proj_weight, dtype=np.float32).T   # [cin, cout]
    pwt = np.ascontiguousarray(
        pwtT.reshape(KT, 128, DIM).transpose(1, 0, 2)).astype(bfloat16)

    bi = np.asarray(b_idx).astype(np.int64)
    qb_all = np.asarray(q_bias, dtype=np.float32)[bi] * np.float32(SCALE)
    vb_all = np.asarray(v_bias, dtype=np.float32)[bi]
    # softmax rows sum to 1, so attn @ (1 x vb) == 1 x vb; push the v bias
    # through the projection into the proj bias
    pb_all = (np.asarray(proj_bias, dtype=np.float32)[bi]
              + vb_all @ np.asarray(proj_weight, dtype=np.float32).T)

    ridx = np.asarray(rel_index).astype(np.int64)
    relE = np.exp(np.asarray(rel_table, dtype=np.float32)[ridx.reshape(-1)]
                  .reshape(N, N, HEADS))           # [n, m, h]
    relM = relE.transpose(1, 0, 2)                  # [m, n, h]
    relt = np.zeros((128, 6, 2, N2), dtype=np.float32)
    for t, (off, mt) in enumerate(TOK_TILES):
        seg = relM[off:off + mt]                    # [mt, n, h]
        relt[0:mt, :, t, :] = (seg.reshape(mt, N, 6, 2)
                               .transpose(0, 2, 3, 1).reshape(mt, 6, N2))
    relt = relt.astype(bfloat16)
    aon = np.ones((65, 128), dtype=bfloat16)
    von = np.ones((128, 12), dtype=bfloat16)

    in_maps = []
    for c in range(NCORES):
        sl = slice(c * BPC, (c + 1) * BPC)
        xs = x[sl]                                  # [8, 197, 768]
        xt = np.ascontiguousarray(
            xs.reshape(NPAIR, 2, N, DIM).transpose(0, 3, 1, 2)
            .reshape(NPAIR, KT, 128, N2).transpose(0, 2, 1, 3))
        qbc = np.ascontiguousarray(
            qb_all[sl].reshape(BPC, KT, 128).transpose(2, 0, 1))
        vpbt = np.ascontiguousarray(
            pb_all[sl].reshape(BPC, KT, 128).transpose(2, 1, 0))
        in_maps.append({
            "xt": xt,
            "wtq": wtq,
            "wtk": wtk,
            "wtv": wtv,
            "pwt": pwt,
            "relt": relt,
            "qbc": qbc,
            "vpbt": vpbt,
            "aon": aon,
            "von": von,
        })
    return in_maps


def _install_ntff_hook():
    """Provide antenv.axon_hooks (absent from this image) so bass_utils can
    capture NTFF profiles through libaxon_pjrt.so, and keep artifacts local."""
    if _CACHE.get("hook_installed"):
        return
    import sys
    import types
    import ctypes
    import contextlib

    so_path = "/opt/axon/libaxon_pjrt.so"
    lib = ctypes.CDLL(so_path)
    lib.axon_start_nrt_profile.argtypes = [
        ctypes.POINTER(ctypes.c_int64),
        ctypes.c_size_t,
    ]
    lib.axon_start_nrt_profile.restype = ctypes.c_int64
    lib.axon_stop_nrt_profile.argtypes = [ctypes.c_char_p]
    lib.axon_stop_nrt_profile.restype = ctypes.c_int64

    @contextlib.contextmanager
    def _hook(output_dir, device_ids):
        import jax

        jax.devices()
        if device_ids:
            ids = (ctypes.c_int64 * len(device_ids))(*device_ids)
            rc = lib.axon_start_nrt_profile(ids, len(device_ids))
        else:
            rc = lib.axon_start_nrt_profile(None, 0)
        if rc != 0:
            raise RuntimeError(f"axon_start_nrt_profile rc={rc}")
        try:
            yield
        finally:
            n = lib.axon_stop_nrt_profile(str(output_dir).encode())
            print(f"ntff profile: {n} file(s) written to {output_dir}")

    mod = types.ModuleType("antenv.axon_hooks")
    mod.get_axon_ntff_profile_hook = lambda: _hook
    mod.set_axon_ntff_profile_hook = lambda h: None
    sys.modules["antenv.axon_hooks"] = mod

    import concourse.bass_utils as bu

    bu.upload_artifacts = lambda tmpdir: str(tmpdir)
    _CACHE["hook_installed"] = True


def kernel(**inputs):
    if "nc" not in _CACHE:
        _CACHE["nc"] = _build_module()
    nc = _CACHE["nc"]

    in_maps = _host_prep(**inputs)
    trace = os.environ.get("KERNEL_TRACE", "0") == "1"
    tmpdir = None
    if trace:
        _install_ntff_hook()
        tmpdir = os.environ.get("KERNEL_TRACE_DIR") or None
    res = run_bass_kernel_spmd(nc, in_maps, core_ids=list(range(NCORES)), trace=trace,
                               tmpdir=tmpdir)
    if trace:
        _CACHE["last_exec_time_ns"] = res.exec_time_ns
        _CACHE["last_results"] = res

    ys = []
    for c in range(NCORES):
        yt = np.asarray(res.results[c]["yt"])       # [4, 6, 128, 394]
        ys.append(yt.reshape(NPAIR, KT, 128, 2, N)
                  .transpose(0, 3, 4, 1, 2).reshape(BPC, N, DIM))
    return np.ascontiguousarray(np.concatenate(ys, axis=0), dtype=np.float32)
